# revision 31
# baseline (speedup 1.0000x reference)
"""Trainium2 Bass kernel for nn_DINONewVq (VQ codebook lookup + softmax probs).

Computes, for z (32,64,32,32) and codebook (4096,64):
  - d_k  = fl(fl(zz + cc_k) - 2*S_k)   (replicates the reference's f32
           rounding order so argmin ties break identically)
  - idx  = argmin_k d (first occurrence)
  - z_q_out = fl(z + fl(codebook[idx] - z))   (straight-through arithmetic,
           bitwise-identical to the reference)
  - q_loss = 1.25 * mean((codebook[idx] - z)^2)
  - distance_prob = softmax(-d/0.5) = exp(2M + 2zz)/rowsum, M = -d

Sharding: data-parallel over the flattened token dim. Each of the 8 cores
takes 4 batches (4096 tokens); the K x D codebook is replicated.
"""

import sys

sys.path.insert(0, "/opt/trn_rl_repo")

import numpy as np

import concourse.bacc as bacc
import concourse.bass as bass
import concourse.mybir as mybir
import concourse.tile as tile
from concourse.bass_utils import run_bass_kernel_spmd

F32 = mybir.dt.float32
F32R = mybir.dt.float32r
U32 = mybir.dt.uint32
I16 = mybir.dt.int16
U16 = mybir.dt.uint16

B, D, H, W = 32, 64, 32, 32
K = 4096
NCORES = 8
NB = B // NCORES          # batches per core
NLOC = NB * H * W         # tokens per core = 4096
NT = NLOC // 128          # token tiles per core = 32
NC512 = K // 512          # K chunks = 8

AF = mybir.ActivationFunctionType
OP = mybir.AluOpType

_CACHE = {}


def build_nc(gather=True, maxidx=True, gp_scale=False, f32r=False):
    nc = bacc.Bacc("TRN2")

    z_in = nc.declare_dram_parameter("z", [NB, D, H * W], F32, isOutput=False)
    cb_in = nc.declare_dram_parameter("codebook", [K, D], F32, isOutput=False)
    ident_in = nc.declare_dram_parameter("ident", [128, 128], F32, isOutput=False)
    probs_out = nc.declare_dram_parameter("probs", [NLOC, K], F32, isOutput=True)
    zq_out = nc.declare_dram_parameter("zq", [NLOC, D], F32, isOutput=True)
    loss_out = nc.declare_dram_parameter("lossp", [128, 1], F32, isOutput=True)

    with tile.TileContext(nc) as tc:
        with (
            tc.tile_pool(name="const", bufs=1) as constp,
            tc.tile_pool(name="cbn", bufs=3) as cbnp,
            tc.tile_pool(name="zt", bufs=3) as ztp,
            tc.tile_pool(name="big", bufs=2) as bigp,
            tc.tile_pool(name="small", bufs=4) as smallp,
            tc.tile_pool(name="fin", bufs=1) as finp,
            tc.tile_pool(name="ptr", bufs=2, space="PSUM") as ptrp,
            tc.tile_pool(name="ps", bufs=3, space="PSUM") as psp,
        ):


            ident = constp.tile([128, 128], F32, tag="ident")
            nc.sync.dma_start(out=ident[:], in_=ident_in[:, :])

            ones64 = constp.tile([64, 1], F32, tag="ones64")
            nc.vector.memset(ones64[:], 1.0)
            ones1 = constp.tile([1, 128], F32, tag="ones1")
            nc.vector.memset(ones1[:], 1.0)

            # cbT2 = 2 * codebook^T (64, K) and negccB = -cc broadcast (128, K),
            # built per 512-column chunk so tile 0 can start early
            cbT2 = constp.tile([64, K], F32, tag="cbT2")
            sqc = constp.tile([64, K], F32, tag="sqc")
            ccrow = constp.tile([1, K], F32, tag="ccrow")
            negccB = constp.tile([128, K], F32, tag="negccB")
            for c in range(NC512):
                sl = slice(c * 512, (c + 1) * 512)
                for k4 in range(4):
                    kc = c * 4 + k4
                    cbn = cbnp.tile([128, 64], F32, tag="cbn")
                    nc.sync.dma_start(out=cbn[:],
                                      in_=cb_in[kc * 128:(kc + 1) * 128, :])
                    pt = ptrp.tile([64, 128], F32, tag="ptr")
                    nc.tensor.transpose(pt[:], cbn[:], ident[:, :])
                    nc.scalar.mul(cbT2[:, kc * 128:(kc + 1) * 128], pt[:], 2.0)
                nc.vector.tensor_tensor(out=sqc[:, sl], in0=cbT2[:, sl],
                                        in1=cbT2[:, sl], op=OP.mult)
                pcc = ptrp.tile([1, 512], F32, tag="ptr")
                nc.tensor.matmul(out=pcc[:], lhsT=ones64[:], rhs=sqc[:, sl],
                                 start=True, stop=True)
                # psum holds 4*cc -> scale by -0.25
                nc.scalar.mul(ccrow[:, sl], pcc[:], -0.25)
                pbc = ptrp.tile([128, 512], F32, tag="ptr")
                nc.tensor.matmul(out=pbc[:], lhsT=ones1[:], rhs=ccrow[:, sl],
                                 start=True, stop=True)
                nc.scalar.copy(negccB[:, sl], pbc[:])

            # z natural-layout accumulator for the whole core (128, NT*64)
            zN_all = finp.tile([128, NT * 64], F32, tag="zN_all")
            idxall = finp.tile([128, NT], U32, tag="idxall")

            for t in range(NT):
                b, j = divmod(t, H * W // 128)
                zT = ztp.tile([64, 128], F32, tag="zT")
                nc.sync.dma_start(out=zT[:], in_=z_in[b, :, j * 128:(j + 1) * 128])

                pzN = ptrp.tile([128, 64], F32, tag="ptr")
                nc.tensor.transpose(pzN[:], zT[:], ident[0:64, 0:64])
                zNs = zN_all[:, t * 64:(t + 1) * 64]
                nc.scalar.copy(zNs, pzN[:])

                sqz = smallp.tile([128, 64], F32, tag="sqz")
                zz = smallp.tile([128, 1], F32, tag="zz")
                nc.vector.tensor_tensor(out=sqz[:], in0=zNs, in1=zNs, op=OP.mult)
                nc.vector.tensor_reduce(out=zz[:], in_=sqz[:],
                                        axis=mybir.AxisListType.X, op=OP.add)
                twozz = smallp.tile([128, 1], F32, tag="twozz")
                nc.scalar.mul(twozz[:], zz[:], 2.0)

                M = bigp.tile([128, K], F32, tag="M")
                E = bigp.tile([128, K], F32, tag="E")
                P = bigp.tile([128, K], F32, tag="P")

                for c in range(4):
                    sl = slice(c * 1024, (c + 1) * 1024)
                    ps = psp.tile([128, 1024], F32, tag="ps2s")
                    nc.tensor.matmul(out=ps[:, 0:512], lhsT=zT[:],
                                     rhs=cbT2[:, c * 1024:c * 1024 + 512],
                                     start=True, stop=True)
                    nc.tensor.matmul(out=ps[:, 512:1024], lhsT=zT[:],
                                     rhs=cbT2[:, c * 1024 + 512:(c + 1) * 1024],
                                     start=True, stop=True)
                    # M = fl(fl(-cc - zz) + 2S)  == -(reference d), exact rounding
                    nc.vector.scalar_tensor_tensor(
                        out=M[:, sl], in0=negccB[:, sl], scalar=zz[:],
                        in1=ps[:], op0=OP.subtract, op1=OP.add)

                # E = exp(2M + 2zz); accum gives the softmax row sum directly
                rowsum = smallp.tile([128, 1], F32, tag="rowsum")
                nc.scalar.activation(out=E[:], in_=M[:], func=AF.Exp,
                                     bias=twozz[:], scale=2.0,
                                     accum_out=rowsum[:])
                recip = smallp.tile([128, 1], F32, tag="recip")
                nc.vector.reciprocal(out=recip[:], in_=rowsum[:])

                # P = E * (1/rowsum)
                nc.scalar.activation(out=P[:], in_=E[:], func=AF.Copy,
                                     scale=recip[:])

                if maxidx:
                    m8 = smallp.tile([128, 8], F32, tag="m8")
                    nc.vector.max(out=m8[:], in_=M[:])
                    i8 = smallp.tile([128, 8], U32, tag="i8")
                    nc.vector.max_index(out=i8[:], in_max=m8[:], in_values=M[:])
                    nc.sync.dma_start(out=idxall[:, t:t + 1], in_=i8[:, 0:1])

                nc.sync.dma_start(out=probs_out[t * 128:(t + 1) * 128, :], in_=P[:])

            # ---- final phase: gather, straight-through output, loss ----
            lossp = finp.tile([128, 1], F32, tag="lossp")
            if gather and maxidx:
                zqall = finp.tile([128, NT * 64], F32, tag="zqall")
                for t in range(NT):
                    nc.gpsimd.indirect_dma_start(
                        out=zqall[:, t * 64:(t + 1) * 64], out_offset=None,
                        in_=cb_in[:, :],
                        in_offset=bass.IndirectOffsetOnAxis(
                            ap=idxall[:, t:t + 1], axis=0))
                diff = finp.tile([128, NT * 64], F32, tag="diff")
                nc.vector.tensor_tensor(out=diff[:], in0=zqall[:], in1=zN_all[:],
                                        op=OP.subtract)
                st = finp.tile([128, NT * 64], F32, tag="st")
                nc.vector.tensor_tensor(out=st[:], in0=zN_all[:], in1=diff[:],
                                        op=OP.add)
                sqd = finp.tile([128, NT * 64], F32, tag="sqd")
                nc.vector.tensor_tensor(out=sqd[:], in0=diff[:], in1=diff[:],
                                        op=OP.mult)
                nc.vector.tensor_reduce(out=lossp[:], in_=sqd[:],
                                        axis=mybir.AxisListType.X, op=OP.add)
                nc.sync.dma_start(
                    out=zq_out[:, :].rearrange("(t p) d -> p t d", p=128),
                    in_=st[:].rearrange("p (t d) -> p t d", t=NT))
            else:
                nc.vector.memset(lossp[:], 0.0)
            nc.sync.dma_start(out=loss_out[:, :], in_=lossp[:])

    nc.compile()
    return nc


def _get_nc():
    if "nc" not in _CACHE:
        _CACHE["nc"] = build_nc()
    return _CACHE["nc"]


def _install_ntff_hook():
    """The agent image's antenv lacks axon_hooks; recreate it so
    run_bass_kernel_spmd(trace=True) can capture NTFF profiles."""
    import types

    try:
        import antenv.axon_hooks  # noqa: F401
        return
    except ImportError:
        pass
    m = types.ModuleType("antenv.axon_hooks")
    m._hook = None
    m.set_axon_ntff_profile_hook = lambda h: setattr(m, "_hook", h)
    m.get_axon_ntff_profile_hook = lambda: m._hook
    sys.modules["antenv.axon_hooks"] = m
    import antenv

    antenv.axon_hooks = m
    from trn_agent_boot.trn_boot import _ntff_profile_via_ctypes

    m._hook = _ntff_profile_via_ctypes("/opt/axon/libaxon_pjrt.so")


def kernel(z, codebook, _trace=False):
    if _trace:
        _install_ntff_hook()
    nc = _get_nc()
    z = np.ascontiguousarray(z, dtype=np.float32)
    codebook = np.ascontiguousarray(codebook, dtype=np.float32)
    ident = np.eye(128, dtype=np.float32)

    in_maps = [
        {
            "z": np.ascontiguousarray(
                z[i * NB:(i + 1) * NB].reshape(NB, D, H * W)),
            "codebook": codebook,
            "ident": ident,
        }
        for i in range(NCORES)
    ]
    r = run_bass_kernel_spmd(nc, in_maps, core_ids=list(range(NCORES)),
                             trace=_trace)
    results = r.results

    probs = np.concatenate([results[i]["probs"] for i in range(NCORES)], axis=0)
    zq_flat = np.concatenate([results[i]["zq"] for i in range(NCORES)], axis=0)
    zq = np.transpose(zq_flat.reshape(B, H, W, D), (0, 3, 1, 2))
    total = np.sum([results[i]["lossp"].astype(np.float64).sum()
                    for i in range(NCORES)])
    q_loss = np.float32(1.25 * total / (B * H * W * D))

    if _trace:
        return (zq, q_loss, probs), r
    return zq, q_loss, probs


# revision 35
# speedup vs baseline: 1.1667x; 1.1667x over previous
"""Trainium2 Bass kernel for nn_DINONewVq (VQ codebook lookup + softmax probs).

Computes, for z (32,64,32,32) and codebook (4096,64):
  - d_k  = fl(fl(zz + cc_k) - 2*S_k)   (replicates the reference's f32
           rounding order so argmin ties break identically)
  - idx  = argmin_k d (first occurrence)
  - z_q_out = fl(z + fl(codebook[idx] - z))   (straight-through arithmetic,
           bitwise-identical to the reference)
  - q_loss = 1.25 * mean((codebook[idx] - z)^2)
  - distance_prob = softmax(-d/0.5) = exp(2M + 2zz)/rowsum, M = -d

Sharding: data-parallel over the flattened token dim. Each of the 8 cores
takes 4 batches (4096 tokens); the K x D codebook is replicated.
"""

import sys

sys.path.insert(0, "/opt/trn_rl_repo")

import numpy as np

import concourse.bacc as bacc
import concourse.bass as bass
import concourse.mybir as mybir
import concourse.tile as tile
from concourse.bass_utils import run_bass_kernel_spmd

F32 = mybir.dt.float32
F32R = mybir.dt.float32r
U32 = mybir.dt.uint32
I16 = mybir.dt.int16
U16 = mybir.dt.uint16

B, D, H, W = 32, 64, 32, 32
K = 4096
NCORES = 8
NB = B // NCORES          # batches per core
NLOC = NB * H * W         # tokens per core = 4096
NT = NLOC // 128          # token tiles per core = 32
NC512 = K // 512          # K chunks = 8

AF = mybir.ActivationFunctionType
OP = mybir.AluOpType

_CACHE = {}


def build_nc(gather=True, maxidx=True, gp_scale=False, f32r=False):
    nc = bacc.Bacc("TRN2")

    z_in = nc.declare_dram_parameter("z", [NB, D, H * W], F32, isOutput=False)
    cb_in = nc.declare_dram_parameter("codebook", [K, D], F32, isOutput=False)
    ident_in = nc.declare_dram_parameter("ident", [128, 128], F32, isOutput=False)
    probs_out = nc.declare_dram_parameter("probs", [NLOC, K], F32, isOutput=True)
    zq_out = nc.declare_dram_parameter("zq", [NLOC, D], F32, isOutput=True)
    loss_out = nc.declare_dram_parameter("lossp", [128, 1], F32, isOutput=True)

    with tile.TileContext(nc) as tc:
        with (
            tc.tile_pool(name="const", bufs=1) as constp,
            tc.tile_pool(name="cbn", bufs=3) as cbnp,
            tc.tile_pool(name="zt", bufs=NT) as ztp,
            tc.tile_pool(name="big", bufs=2) as bigp,
            tc.tile_pool(name="small", bufs=4) as smallp,
            tc.tile_pool(name="fin", bufs=1) as finp,
            tc.tile_pool(name="ptr", bufs=2, space="PSUM") as ptrp,
            tc.tile_pool(name="ps", bufs=3, space="PSUM") as psp,
        ):


            ident = constp.tile([128, 128], F32, tag="ident")
            nc.sync.dma_start(out=ident[:], in_=ident_in[:, :])

            ones64 = constp.tile([64, 1], F32, tag="ones64")
            nc.vector.memset(ones64[:], 1.0)
            ones1 = constp.tile([1, 128], F32, tag="ones1")
            nc.vector.memset(ones1[:], 1.0)

            # cbT2 = 2 * codebook^T (64, K) and negccB = -cc broadcast (128, K),
            # built per 512-column chunk so tile 0 can start early
            cbT2 = constp.tile([64, K], F32, tag="cbT2")
            sqc = bigp.tile([64, K], F32, tag="M")
            negccB = constp.tile([128, K], F32, tag="negccB")
            for c in range(NC512):
                sl = slice(c * 512, (c + 1) * 512)
                for k4 in range(4):
                    kc = c * 4 + k4
                    cbn = cbnp.tile([128, 64], F32, tag="cbn")
                    nc.sync.dma_start(out=cbn[:],
                                      in_=cb_in[kc * 128:(kc + 1) * 128, :])
                    pt = ptrp.tile([64, 128], F32, tag="ptr")
                    nc.tensor.transpose(pt[:], cbn[:], ident[:, :])
                    nc.scalar.mul(cbT2[:, kc * 128:(kc + 1) * 128], pt[:], 2.0)
                nc.vector.tensor_tensor(out=sqc[:, sl], in0=cbT2[:, sl],
                                        in1=cbT2[:, sl], op=OP.mult)
                pcc = psp.tile([1, 512], F32, tag="ps2s")
                nc.tensor.matmul(out=pcc[:], lhsT=ones64[:], rhs=sqc[:, sl],
                                 start=True, stop=True)
                # psum holds 4*cc -> scale by -0.25
                ccrow = smallp.tile([1, 512], F32, tag="ccrow")
                nc.scalar.mul(ccrow[:], pcc[:], -0.25)
                pbc = psp.tile([128, 512], F32, tag="ps2s")
                nc.tensor.matmul(out=pbc[:], lhsT=ones1[:], rhs=ccrow[:],
                                 start=True, stop=True)
                nc.scalar.copy(negccB[:, sl], pbc[:])

            # prepass: load all z tiles, build token-major z, and row norms
            zN_all = finp.tile([128, NT * 64], F32, tag="zN_all")
            idxall = finp.tile([128, NT], U32, tag="idxall")
            zzall = finp.tile([128, NT], F32, tag="zzall")
            zts = []
            for t in range(NT):
                b, j = divmod(t, H * W // 128)
                zT = ztp.tile([64, 128], F32, tag="zT")
                zts.append(zT)
                nc.sync.dma_start(out=zT[:], in_=z_in[b, :, j * 128:(j + 1) * 128])
                pzN = ptrp.tile([128, 64], F32, tag="ptr")
                nc.tensor.transpose(pzN[:], zT[:], ident[0:64, 0:64])
                zNs = zN_all[:, t * 64:(t + 1) * 64]
                nc.scalar.copy(zNs, pzN[:])
                sqz = smallp.tile([128, 64], F32, tag="sqz")
                nc.vector.tensor_tensor(out=sqz[:], in0=zNs, in1=zNs, op=OP.mult)
                nc.vector.tensor_reduce(out=zzall[:, t:t + 1], in_=sqz[:],
                                        axis=mybir.AxisListType.X, op=OP.add)

            for t in range(NT):
                zT = zts[t]
                zz = zzall[:, t:t + 1]
                twozz = smallp.tile([128, 1], F32, tag="twozz")
                nc.scalar.mul(twozz[:], zz, 2.0)

                M = bigp.tile([128, K], F32, tag="M")
                E = bigp.tile([128, K], F32, tag="E")
                P = bigp.tile([128, K], F32, tag="P")
                sumparts = smallp.tile([128, 2], F32, tag="sumparts")

                for c in range(4):
                    sl = slice(c * 1024, (c + 1) * 1024)
                    ps = psp.tile([128, 1024], F32, tag="ps2s")
                    nc.tensor.matmul(out=ps[:, 0:512], lhsT=zT[:],
                                     rhs=cbT2[:, c * 1024:c * 1024 + 512],
                                     start=True, stop=True)
                    nc.tensor.matmul(out=ps[:, 512:1024], lhsT=zT[:],
                                     rhs=cbT2[:, c * 1024 + 512:(c + 1) * 1024],
                                     start=True, stop=True)
                    # M = fl(fl(-cc - zz) + 2S)  == -(reference d), exact rounding
                    nc.vector.scalar_tensor_tensor(
                        out=M[:, sl], in0=negccB[:, sl], scalar=zz,
                        in1=ps[:], op0=OP.subtract, op1=OP.add)

                for c in range(2):
                    sl = slice(c * 2048, (c + 1) * 2048)
                    # E = exp(2M + 2zz) = exp(4S - 2cc +- quantization)
                    nc.scalar.activation(
                        out=E[:, sl], in_=M[:, sl], func=AF.Exp,
                        bias=twozz[:], scale=2.0,
                        accum_out=sumparts[:, c:c + 1])

                rowsum = smallp.tile([128, 1], F32, tag="rowsum")
                nc.vector.tensor_reduce(out=rowsum[:], in_=sumparts[:],
                                        axis=mybir.AxisListType.X, op=OP.add)
                recip = smallp.tile([128, 1], F32, tag="recip")
                nc.vector.reciprocal(out=recip[:], in_=rowsum[:])

                # P = E * (1/rowsum)
                nc.scalar.activation(out=P[:], in_=E[:], func=AF.Copy,
                                     scale=recip[:])

                if maxidx:
                    m8 = smallp.tile([128, 8], F32, tag="m8")
                    nc.vector.max(out=m8[:], in_=M[:])
                    i8 = smallp.tile([128, 8], U32, tag="i8")
                    nc.vector.max_index(out=i8[:], in_max=m8[:], in_values=M[:])
                    nc.sync.dma_start(out=idxall[:, t:t + 1], in_=i8[:, 0:1])

                nc.sync.dma_start(out=probs_out[t * 128:(t + 1) * 128, :], in_=P[:])

            # ---- final phase: gather, straight-through output, loss ----
            lossp = finp.tile([128, 1], F32, tag="lossp")
            if gather and maxidx:
                zqall = finp.tile([128, NT * 64], F32, tag="zqall")
                for t in range(NT):
                    nc.gpsimd.indirect_dma_start(
                        out=zqall[:, t * 64:(t + 1) * 64], out_offset=None,
                        in_=cb_in[:, :],
                        in_offset=bass.IndirectOffsetOnAxis(
                            ap=idxall[:, t:t + 1], axis=0))
                diff = finp.tile([128, NT * 64], F32, tag="diff")
                nc.vector.tensor_tensor(out=diff[:], in0=zqall[:], in1=zN_all[:],
                                        op=OP.subtract)
                st = finp.tile([128, NT * 64], F32, tag="st")
                nc.vector.tensor_tensor(out=st[:], in0=zN_all[:], in1=diff[:],
                                        op=OP.add)
                # reuse zqall (dead after diff) for the squared residuals
                nc.vector.tensor_tensor(out=zqall[:], in0=diff[:], in1=diff[:],
                                        op=OP.mult)
                nc.vector.tensor_reduce(out=lossp[:], in_=zqall[:],
                                        axis=mybir.AxisListType.X, op=OP.add)
                nc.sync.dma_start(
                    out=zq_out[:, :].rearrange("(t p) d -> p t d", p=128),
                    in_=st[:].rearrange("p (t d) -> p t d", t=NT))
            else:
                nc.vector.memset(lossp[:], 0.0)
            nc.sync.dma_start(out=loss_out[:, :], in_=lossp[:])

    nc.compile()
    return nc


def _get_nc():
    if "nc" not in _CACHE:
        _CACHE["nc"] = build_nc()
    return _CACHE["nc"]


def _install_ntff_hook():
    """The agent image's antenv lacks axon_hooks; recreate it so
    run_bass_kernel_spmd(trace=True) can capture NTFF profiles."""
    import types

    try:
        import antenv.axon_hooks  # noqa: F401
        return
    except ImportError:
        pass
    m = types.ModuleType("antenv.axon_hooks")
    m._hook = None
    m.set_axon_ntff_profile_hook = lambda h: setattr(m, "_hook", h)
    m.get_axon_ntff_profile_hook = lambda: m._hook
    sys.modules["antenv.axon_hooks"] = m
    import antenv

    antenv.axon_hooks = m
    from trn_agent_boot.trn_boot import _ntff_profile_via_ctypes

    m._hook = _ntff_profile_via_ctypes("/opt/axon/libaxon_pjrt.so")


def kernel(z, codebook, _trace=False):
    if _trace:
        _install_ntff_hook()
    nc = _get_nc()
    z = np.ascontiguousarray(z, dtype=np.float32)
    codebook = np.ascontiguousarray(codebook, dtype=np.float32)
    ident = np.eye(128, dtype=np.float32)

    in_maps = [
        {
            "z": np.ascontiguousarray(
                z[i * NB:(i + 1) * NB].reshape(NB, D, H * W)),
            "codebook": codebook,
            "ident": ident,
        }
        for i in range(NCORES)
    ]
    r = run_bass_kernel_spmd(nc, in_maps, core_ids=list(range(NCORES)),
                             trace=_trace)
    results = r.results

    probs = np.concatenate([results[i]["probs"] for i in range(NCORES)], axis=0)
    zq_flat = np.concatenate([results[i]["zq"] for i in range(NCORES)], axis=0)
    zq = np.transpose(zq_flat.reshape(B, H, W, D), (0, 3, 1, 2))
    total = np.sum([results[i]["lossp"].astype(np.float64).sum()
                    for i in range(NCORES)])
    q_loss = np.float32(1.25 * total / (B * H * W * D))

    if _trace:
        return (zq, q_loss, probs), r
    return zq, q_loss, probs
